# revision 5
# baseline (speedup 1.0000x reference)
"""Multi-head self-attention Trainium2 kernel (8 NeuronCores, SPMD).

Problem: x[B=4,N=2048,H=16,D=64], per-head Wq/Wk/Wv/Wo[H,D,D]+biases.
The computation is fully independent per (b,h) pair: 64 problems, 8/core.

Per-problem device layout (everything "transposed", i on free dim):
  xT_aug [65,2048]  = [x(b,:,h,:).T ; ones]          (host-prepped, bf16)
  qT = (Wq/32 | bq/32)-augmented proj -> [64,2048]
  kT likewise (unscaled), v natural [2048,64] via 16 small matmuls
  sT[j,i] = sum_d kT[d,j] qT[d,i]   (j-tiles of [128, i-half 1024])
  p = exp(sT)  (no max subtraction: |s|<~1, softmax shift-invariant)
  attn_ext[f,i] = sum_j [v|1][j,f] p[j,i]  -> rows 0..63 attnT, row 64 sums
  out = Wo_aug^T @ [attnT*r ; ones], r = 1/sums  -> [64,2048] -> host .T

Two problems are interleaved (ping-pong) through the j-loops so every
engine always has independent work queued: this environment has high
cross-engine semaphore latency, and the P@V matmuls are emitted with a
one-step lag behind exp so the PE never stalls at its queue head.
"""

import numpy as np
import ml_dtypes

import concourse.bass as bass
import concourse.bacc as bacc
import concourse.mybir as mybir
from concourse.tile import TileContext
from concourse import bass_utils

B, N, H, D = 4, 2048, 16, 64
NCORES = 8
PPC = 8  # problems (b,h pairs) per core
DA = D + 1  # augmented (bias/ones) row count
JT = N // 128  # 16 j-tiles
NH = N // 2  # half of i dimension (PSUM tiling)

F32 = mybir.dt.float32
BF16 = mybir.dt.bfloat16
EXP = mybir.ActivationFunctionType.Exp

_cache = {}


def _build(loop_n=1):
    if loop_n in _cache:
        return _cache[loop_n]
    nc = bacc.Bacc("TRN2", target_bir_lowering=False, debug=False, num_devices=NCORES)
    xt = nc.dram_tensor("xt", [PPC, DA, N], BF16, kind="ExternalInput")
    wt = nc.dram_tensor("wt", [DA, PPC * 4 * D], BF16, kind="ExternalInput")
    ot = nc.dram_tensor("ot", [PPC, D, N], F32, kind="ExternalOutput")

    with TileContext(nc) as tc:
        with (
            tc.tile_pool(name="w", bufs=1) as pw,
            tc.tile_pool(name="x", bufs=3) as px,
            tc.tile_pool(name="qk", bufs=6) as pqk,
            tc.tile_pool(name="v", bufs=3) as pv,
            tc.tile_pool(name="pt", bufs=6) as ppt,
            tc.tile_pool(name="misc", bufs=4) as pm,
            tc.tile_pool(name="out", bufs=3) as po,
            tc.tile_pool(name="ps_sc", bufs=2, space="PSUM") as ps_sc,
            tc.tile_pool(name="ps_att", bufs=2, space="PSUM") as ps_att,
        ):
            w_all = pw.tile([DA, PPC * 4 * D], BF16, tag="w")
            nc.sync.dma_start(w_all[:], wt.ap())

            def proj(s):
                """Load x, compute qT/kT [64,N] and v_aug [128, 16*65] bf16."""
                woff = s * 4 * D
                xa = px.tile([DA, N], BF16, tag="x", name=f"xa{s}")
                nc.sync.dma_start(xa[:], xt.ap()[s])

                qk = []
                for m in range(2):
                    t_sb = pqk.tile([D, N], BF16, tag="qk", name=f"qk{m}_{s}")
                    for h in range(2):
                        ps = ps_sc.tile([D, NH], F32, tag="sc", name="ps_p")
                        for c in range(2):
                            nc.tensor.matmul(
                                ps[:, c * 512 : (c + 1) * 512],
                                w_all[:, woff + m * D : woff + (m + 1) * D],
                                xa[:, h * NH + c * 512 : h * NH + (c + 1) * 512],
                                start=True,
                                stop=True,
                            )
                        nc.vector.tensor_copy(t_sb[:, h * NH : (h + 1) * NH], ps[:])
                    qk.append(t_sb)

                v_ps = ps_sc.tile([128, N // 2], F32, tag="sc", name="v_ps")
                for jt in range(JT):
                    nc.tensor.matmul(
                        v_ps[:, jt * D : (jt + 1) * D],
                        xa[:, jt * 128 : (jt + 1) * 128],
                        w_all[:, woff + 2 * D : woff + 3 * D],
                        start=True,
                        stop=True,
                    )
                v_aug = pv.tile([128, JT * (D + 1)], BF16, tag="v", name=f"v{s}")
                nc.gpsimd.memset(v_aug[:], 1.0)
                nc.vector.tensor_copy(
                    v_aug.rearrange("p (t c) -> p t c", c=D + 1)[:, :, 0:D],
                    v_ps.rearrange("p (t c) -> p t c", c=D),
                )
                return qk[0], qk[1], v_aug

            def tail(s, h, att_ps, o_sb):
                """Project att half to output half, normalizing at the end.

                out = (Wo^T @ attnT_raw + bo*sums) * r = attnT_n @ Wo + bo,
                so no normalization or ones-row is needed before the matmul.
                """
                woff = s * 4 * D
                a_bf = pm.tile([DA, NH], BF16, tag="abf", name=f"abf{s}_{h}")
                nc.vector.tensor_copy(a_bf[:], att_ps[:])
                r = pm.tile([1, NH], F32, tag="r", name=f"r{s}_{h}")
                nc.vector.reciprocal(r[:], att_ps[D : D + 1, :])
                r_b = pm.tile([D, NH], F32, tag="rb", name=f"rb{s}_{h}")
                nc.gpsimd.partition_broadcast(r_b[:], r[:])
                ops = ps_sc.tile([D, NH], F32, tag="sc", name="ops")
                for c in range(2):
                    nc.tensor.matmul(
                        ops[:, c * 512 : (c + 1) * 512],
                        w_all[:, woff + 3 * D : woff + 4 * D],
                        a_bf[:, c * 512 : (c + 1) * 512],
                        start=True,
                        stop=True,
                    )
                nc.vector.tensor_mul(o_sb[:, h * NH : (h + 1) * NH], ops[:], r_b[:])

            def pair(sa, sb):
                """Interleaved attention for problems sa, sb."""
                ctx = {}
                for s in (sa, sb):
                    qT, kT, v_aug = proj(s)
                    o_sb = po.tile([D, N], F32, tag="o", name=f"o{s}")
                    ctx[s] = (qT, kT, v_aug, o_sb)

                for h in range(2):
                    att = {
                        s: ps_att.tile([DA, NH], F32, tag="att", name=f"att{s}_{h}")
                        for s in (sa, sb)
                    }
                    sps = {}
                    pts = {}

                    def sc_exp(s, jt):
                        qT, kT, v_aug, _ = ctx[s]
                        sp = ps_sc.tile([128, NH], F32, tag="sc", name="sps")
                        for c in range(2):
                            nc.tensor.matmul(
                                sp[:, c * 512 : (c + 1) * 512],
                                kT[:, jt * 128 : (jt + 1) * 128],
                                qT[:, h * NH + c * 512 : h * NH + (c + 1) * 512],
                                start=True,
                                stop=True,
                            )
                        pt = ppt.tile([128, NH], BF16, tag="pt", name="pt")
                        nc.scalar.activation(pt[:], sp[:], EXP)
                        pts[s] = pt

                    def att_mm(s, jt):
                        _, _, v_aug, _ = ctx[s]
                        for c in range(2):
                            nc.tensor.matmul(
                                att[s][:, c * 512 : (c + 1) * 512],
                                v_aug[:, jt * (D + 1) : (jt + 1) * (D + 1)],
                                pts[s][:, c * 512 : (c + 1) * 512],
                                start=(jt == 0),
                                stop=(jt == JT - 1),
                            )

                    # software pipeline: att lags sc/exp by one jt step
                    for jt in range(JT):
                        sc_exp(sa, jt)
                        if jt > 0:
                            att_mm(sb, jt - 1)
                        sc_exp(sb, jt)
                        att_mm(sa, jt)
                    att_mm(sb, JT - 1)

                    for s in (sa, sb):
                        tail(s, h, att[s], ctx[s][3])

                for s in (sa, sb):
                    nc.sync.dma_start(ot.ap()[s], ctx[s][3][:])

            def body():
                for sp in range(PPC // 2):
                    pair(2 * sp, 2 * sp + 1)

            if loop_n > 1:
                with tc.For_i(0, loop_n, 1):
                    body()
            else:
                body()

    nc.compile()
    _cache[loop_n] = nc
    return nc


def _host_prep(x, Wq, bq, Wk, bk, Wv, bv, Wo, bo):
    """Returns per-core in_maps."""
    x = np.asarray(x, np.float32)
    scale = 1.0 / np.sqrt(np.float32(H * D))
    in_maps = []
    for c in range(NCORES):
        xt = np.empty((PPC, DA, N), ml_dtypes.bfloat16)
        wt = np.empty((DA, PPC * 4 * D), np.float32)
        for s in range(PPC):
            p = c * PPC + s
            b, h = divmod(p, H)
            xt[s, :D, :] = x[b, :, h, :].T.astype(ml_dtypes.bfloat16)
            xt[s, D, :] = 1.0
            o = s * 4 * D
            wt[:D, o : o + D] = Wq[h] * scale
            wt[D, o : o + D] = bq[h] * scale
            wt[:D, o + D : o + 2 * D] = Wk[h]
            wt[D, o + D : o + 2 * D] = bk[h]
            wt[:D, o + 2 * D : o + 3 * D] = Wv[h]
            wt[D, o + 2 * D : o + 3 * D] = bv[h]
            wt[:D, o + 3 * D : o + 4 * D] = Wo[h]
            wt[D, o + 3 * D : o + 4 * D] = bo[h]
        in_maps.append({"xt": xt, "wt": wt.astype(ml_dtypes.bfloat16)})
    return in_maps


def _gather(results):
    out = np.empty((B, N, H, D), np.float32)
    for c in range(NCORES):
        ot = results[c]["ot"]
        for s in range(PPC):
            b, h = divmod(c * PPC + s, H)
            out[b, :, h, :] = ot[s].T
    return out


def run(in_maps, loop_n=1, **kw):
    nc = _build(loop_n)
    return bass_utils.run_bass_kernel_spmd(
        nc, in_maps, core_ids=list(range(NCORES)), **kw
    )


def kernel(x, Wq, bq, Wk, bk, Wv, bv, Wo, bo):
    in_maps = _host_prep(x, Wq, bq, Wk, bk, Wv, bv, Wo, bo)
    res = run(in_maps)
    return _gather(res.results)
